# revision 23
# baseline (speedup 1.0000x reference)
"""Trainium2 Bass kernel for causal multi-head attention.

Shapes (hardcoded): B=4, T=2048, D=1024, H=16, Dh=64, fp32 I/O.

Strategy (8 NeuronCores, tensor-parallel over heads):
  - Each core c owns heads (2c, 2c+1): computes Q^T/K^T/V projections for its
    128 head-dims over the whole [B*T, D] input (contracting D on the PE),
    then causal flash-style attention in "scores-transposed" orientation
    (S^T[k, q] blocks) so softmax needs no on-chip transposes:
      * exp on ScalarE (no max subtraction: logits are O(+-4) by construction)
      * denominator via an appended ones-column in the V stationary operand
        (partition-axis reduction done by the PE itself)
      * division folded into the PSUM->SBUF copy against a PE-broadcast
        reciprocal
  - An on-device AllToAll re-shards ctx^T from head-sharded to row-sharded,
    then each core computes out rows [1024c : 1024c+1024) = ctx @ Wo + bo.
  - Host side only slices/cats and casts dtypes.

All matmul operands are fp16 (same PE throughput as bf16, 3 extra mantissa
bits); all accumulation is fp32 in PSUM.
"""

import sys

sys.path.insert(0, "/opt/trn_rl_repo")

import numpy as np

import concourse.bass as bass
import concourse.mybir as mybir
import concourse.tile as tile
from concourse import bacc
from concourse import bass_utils

N_CORES = 8
B, T, D, H, DH = 4, 2048, 1024, 16, 64
BT = B * T  # 8192
KS = D // 128  # 8 contraction subtiles
TC = 512  # t-chunk for projections
NTC = BT // TC  # 16
QC = 512  # query chunk in attention
NQC = T // QC  # 4 per batch
KB = 128  # key block
NKB = T // KB  # 16 per batch
ROWS = BT // N_CORES  # 1024 out rows per core

F16 = mybir.dt.float16
F32 = mybir.dt.float32

_CACHE = {}


def _build():
    nc = bacc.Bacc("TRN2", target_bir_lowering=False, num_devices=N_CORES)

    x_d = nc.dram_tensor("x", [D, BT], F16, kind="ExternalInput")  # pre-transposed
    wq_d = nc.dram_tensor("wq", [D, 128], F16, kind="ExternalInput")
    wk_d = nc.dram_tensor("wk", [D, 128], F16, kind="ExternalInput")
    wv_d = nc.dram_tensor("wv", [D, 128], F16, kind="ExternalInput")
    wo_d = nc.dram_tensor("wo", [D, D], F16, kind="ExternalInput")
    bo_d = nc.dram_tensor("bo", [D], F32, kind="ExternalInput")
    e2_d = nc.dram_tensor("e2", [2, 128], F16, kind="ExternalInput")
    cmask_d = nc.dram_tensor("cmask", [4, 128, QC], F16, kind="ExternalInput")
    out_d = nc.dram_tensor("out", [B, ROWS // B, D], F32, kind="ExternalOutput")

    with tile.TileContext(nc) as tc:
        with (
            tc.tile_pool(name="persist", bufs=1) as persist,
            tc.tile_pool(name="xt", bufs=3) as xtp,
            tc.tile_pool(name="work", bufs=12) as work,
            tc.tile_pool(name="tail", bufs=2) as tailp,
            tc.tile_pool(name="ctx", bufs=3) as ctxp,
            tc.tile_pool(name="outp", bufs=3) as outp,
            tc.tile_pool(name="ps_proj", bufs=1, space="PSUM") as ps_proj,
            tc.tile_pool(name="ps_s", bufs=4, space="PSUM") as ps_s,
            tc.tile_pool(name="ps_av", bufs=2, space="PSUM") as ps_av,
            tc.tile_pool(name="ps_rb", bufs=1, space="PSUM") as ps_rb,
            tc.tile_pool(name="dram", bufs=1, space="DRAM") as dram,
        ):
            # ---- persistent state ----
            wq_sb = persist.tile([128, KS, 128], F16)
            wk_sb = persist.tile([128, KS, 128], F16)
            wv_sb = persist.tile([128, KS, 128], F16)
            wo_sb = persist.tile([128, KS, D], F16)
            nc.sync.dma_start(wq_sb[:], wq_d.rearrange("(o p) h -> p o h", p=128))
            nc.sync.dma_start(wk_sb[:], wk_d.rearrange("(o p) h -> p o h", p=128))
            nc.sync.dma_start(wv_sb[:], wv_d.rearrange("(o p) h -> p o h", p=128))
            nc.sync.dma_start(wo_sb[:], wo_d.rearrange("(r p) n -> p r n", p=128))

            qt_sb = persist.tile([128, BT], F16)  # [2 heads x 64, global t]
            kt_sb = persist.tile([128, BT], F16)
            v0_sb = persist.tile([128, B * NKB, DH + 1], F16)  # + ones col
            v1_sb = persist.tile([128, B * NKB, DH + 1], F16)
            nc.vector.memset(v0_sb[:, :, DH : DH + 1], 1.0)
            nc.vector.memset(v1_sb[:, :, DH : DH + 1], 1.0)

            # bias broadcast [128, D] fp32 via PE ones-trick
            ones_col = persist.tile([1, 128], F32)
            nc.vector.memset(ones_col[:], 1.0)
            bo_sb = persist.tile([1, D], F32)
            nc.sync.dma_start(bo_sb[:], bo_d[None, :])
            bias_sb = persist.tile([128, D], F32)
            for nch in range(2):
                bps = ps_proj.tile([128, 512], F32, tag="proj")
                nc.tensor.matmul(
                    bps[:], ones_col[:], bo_sb[:, nch * 512 : (nch + 1) * 512]
                )
                nc.vector.tensor_copy(bias_sb[:, nch * 512 : (nch + 1) * 512], bps[:])

            # E2 selector for reciprocal broadcast: rows 0-63 <- r2[0], 64-127 <- r2[1]
            e2_sb = persist.tile([2, 128], F16)
            nc.sync.dma_start(e2_sb[:], e2_d[:])

            # diagonal causal masks: mask_i[p, j] = 1 if j >= p + i*128 else 0
            cmask_sb = persist.tile([128, 4, QC], F16)
            nc.sync.dma_start(cmask_sb[:], cmask_d.rearrange("i p j -> p i j"))
            masks = [cmask_sb[:, i, :] for i in range(4)]

            # ---- phase 1: projections ----
            for tcn in range(NTC):
                t0 = tcn * TC
                xt = xtp.tile([128, KS, TC], F16, tag="xt")
                nc.sync.dma_start(
                    xt[:],
                    x_d[:, t0 : t0 + TC].rearrange("(o p) t -> p o t", p=128),
                )
                for w_sb, dst in ((wq_sb, qt_sb), (wk_sb, kt_sb)):
                    pp = ps_proj.tile([128, TC], F32, tag="proj")
                    for ks in range(KS):
                        nc.tensor.matmul(
                            pp[:],
                            w_sb[:, ks, :],
                            xt[:, ks, :],
                            start=(ks == 0),
                            stop=(ks == KS - 1),
                        )
                    nc.vector.tensor_copy(dst[:, t0 : t0 + TC], pp[:])
                for sub in range(TC // 128):
                    vp_full = ps_proj.tile([128, TC], F32, tag="proj", name="vp")
                    vp = vp_full[:, :128]
                    for ks in range(KS):
                        nc.tensor.matmul(
                            vp[:],
                            xt[:, ks, sub * 128 : (sub + 1) * 128],
                            wv_sb[:, ks, :],
                            start=(ks == 0),
                            stop=(ks == KS - 1),
                        )
                    kbg = tcn * (TC // 128) + sub
                    nc.vector.tensor_copy(v0_sb[:, kbg, 0:DH], vp[:, 0:DH])
                    nc.vector.tensor_copy(v1_sb[:, kbg, 0:DH], vp[:, DH : 2 * DH])

            # ---- phase 2: attention (scores-transposed flash) ----
            RB4 = ROWS // B  # 256 out rows per core per batch
            cc_ins = [dram.tile([N_CORES, 128, RB4], F16, name=f"cc_in{b}", tag=f"cc_in{b}") for b in range(B)]
            cc_outs = [dram.tile([N_CORES, 128, RB4], F16, name=f"cc_out{b}", tag=f"cc_out{b}") for b in range(B)]
            ao_sbs = []

            for b in range(B):
                for qc in range(NQC):
                    q0 = b * T + qc * QC
                    nkb = 4 * qc + 4
                    av0_full = ps_av.tile([128, QC], F32, tag="av", name="av0")
                    av1_full = ps_av.tile([128, QC], F32, tag="av", name="av1")
                    av0 = av0_full[: DH + 1]
                    av1 = av1_full[: DH + 1]
                    for kb in range(nkb):
                        k0 = b * T + kb * KB
                        kbg = b * NKB + kb
                        first, last = kb == 0, kb == nkb - 1
                        for h, av in ((0, av0), (1, av1)):
                            hs = slice(h * 64, (h + 1) * 64)
                            sp = ps_s.tile([128, QC], F32, tag="s")
                            nc.tensor.matmul(
                                sp[:],
                                kt_sb[hs, k0 : k0 + KB],
                                qt_sb[hs, q0 : q0 + QC],
                            )
                            e = work.tile([128, QC], F16, tag="e")
                            nc.scalar.activation(
                                e[:], sp[:], mybir.ActivationFunctionType.Exp,
                                scale=0.125,
                            )
                            if kb >= 4 * qc:
                                nc.vector.tensor_tensor(
                                    e[:], e[:], masks[kb - 4 * qc],
                                    mybir.AluOpType.mult,
                                )
                            vsb = v0_sb if h == 0 else v1_sb
                            nc.tensor.matmul(
                                av[:], vsb[:, kbg, :], e[:],
                                start=first, stop=last,
                            )
                    # reciprocal of denominators (row 64 of each av bank)
                    u0 = tailp.tile([DH + 1, QC], F32, tag="u0")
                    u1 = tailp.tile([DH + 1, QC], F32, tag="u1")
                    nc.vector.tensor_copy(u0[:], av0[:])
                    nc.vector.tensor_copy(u1[:], av1[:])
                    d2a = tailp.tile([1, QC], F32, tag="d2a")
                    d2b = tailp.tile([1, QC], F32, tag="d2b")
                    nc.vector.tensor_copy(d2a[:], av0[DH : DH + 1, :])
                    nc.vector.tensor_copy(d2b[:], av1[DH : DH + 1, :])
                    r2 = tailp.tile([2, QC], F32, tag="r2")
                    r1t = tailp.tile([1, QC], F32, tag="r1t")
                    nc.vector.reciprocal_approx_fast(r2[0:1, :], d2a[:])
                    nc.vector.reciprocal_approx_fast(r1t[:], d2b[:])
                    nc.sync.dma_start(r2[1:2, :], r1t[:])
                    r2h = tailp.tile([2, QC], F16, tag="r2h")
                    nc.vector.tensor_copy(r2h[:], r2[:])
                    rb = ps_rb.tile([128, QC], F32, tag="rb")
                    nc.tensor.matmul(rb[:], e2_sb[:], r2h[:])
                    ctx2 = ctxp.tile([128, QC], F16, tag="ctx")
                    nc.vector.tensor_tensor(
                        ctx2[0:64, :], u0[0:64, :], rb[0:64, :],
                        mybir.AluOpType.mult,
                    )
                    nc.vector.tensor_tensor(
                        ctx2[64:128, :], u1[0:64, :], rb[64:128, :],
                        mybir.AluOpType.mult,
                    )
                    s0 = qc * QC // RB4  # first shard covered by this chunk
                    nc.sync.dma_start(
                        cc_ins[b][s0 : s0 + QC // RB4].rearrange("s p f -> p s f"),
                        ctx2[:].rearrange("p (s f) -> p s f", s=QC // RB4),
                    )

                # ---- per-batch all-to-all + output projection ----
                nc.gpsimd.collective_compute(
                    "AllToAll",
                    mybir.AluOpType.bypass,
                    replica_groups=[list(range(N_CORES))],
                    ins=[cc_ins[b][:]],
                    outs=[cc_outs[b][:]],
                )
                ao_sb = persist.tile([128, KS, RB4], F16, name=f"ao{b}", tag=f"ao{b}")
                ao_sbs.append(ao_sb)
                nc.sync.dma_start(ao_sb[:], cc_outs[b].rearrange("r p t -> p r t"))
                for mb in range(RB4 // 128):
                    for nch in range(2):
                        op = ps_proj.tile([128, 512], F32, tag="proj")
                        for r in range(KS):
                            nc.tensor.matmul(
                                op[:],
                                ao_sb[:, r, mb * 128 : (mb + 1) * 128],
                                wo_sb[:, r, nch * 512 : (nch + 1) * 512],
                                start=(r == 0),
                                stop=(r == KS - 1),
                            )
                        osb = outp.tile([128, 512], F32, tag="osb")
                        nc.vector.tensor_tensor(
                            osb[:], op[:], bias_sb[:, nch * 512 : (nch + 1) * 512],
                            mybir.AluOpType.add,
                        )
                        nc.sync.dma_start(
                            out_d[b, mb * 128 : (mb + 1) * 128,
                                  nch * 512 : (nch + 1) * 512],
                            osb[:],
                        )

    nc.compile()
    return nc


def _get_nc():
    if "nc" not in _CACHE:
        _CACHE["nc"] = _build()
    return _CACHE["nc"]


def prepare_in_maps(x, Wq, Wk, Wv, Wo, bo):
    x16 = np.ascontiguousarray(np.asarray(x, dtype=np.float32).reshape(BT, D).T).astype(np.float16)
    wo16 = np.asarray(Wo, dtype=np.float32).astype(np.float16)
    bo32 = np.ascontiguousarray(np.asarray(bo, dtype=np.float32))
    e2 = np.zeros((2, 128), dtype=np.float16)
    e2[0, 0:64] = 1.0
    e2[1, 64:128] = 1.0
    cmask = np.zeros((4, 128, QC), dtype=np.float16)
    for i in range(4):
        p = np.arange(128)[:, None]
        j = np.arange(QC)[None, :]
        cmask[i] = (j >= p + i * 128).astype(np.float16)
    in_maps = []
    for c in range(N_CORES):
        cs = slice(128 * c, 128 * (c + 1))
        in_maps.append(
            {
                "x": x16,
                "wq": np.ascontiguousarray(np.asarray(Wq, np.float32)[:, cs]).astype(np.float16),
                "wk": np.ascontiguousarray(np.asarray(Wk, np.float32)[:, cs]).astype(np.float16),
                "wv": np.ascontiguousarray(np.asarray(Wv, np.float32)[:, cs]).astype(np.float16),
                "wo": wo16,
                "bo": bo32,
                "e2": e2,
                "cmask": cmask,
            }
        )
    return in_maps


def kernel(x, Wq, Wk, Wv, Wo, bo, _trace=False):
    nc = _get_nc()
    in_maps = prepare_in_maps(x, Wq, Wk, Wv, Wo, bo)
    res = bass_utils.run_bass_kernel_spmd(
        nc, in_maps, list(range(N_CORES)), trace=_trace
    )
    if _trace:
        _CACHE["last_results"] = res
    out = np.empty((B, T, D), dtype=np.float32)
    rb4 = ROWS // B
    for c in range(N_CORES):
        oc = res.results[c]["out"]  # [B, 256, D]
        for b in range(B):
            out[b, rb4 * c : rb4 * (c + 1), :] = oc[b]
    return out


# revision 24
# speedup vs baseline: 1.0120x; 1.0120x over previous
"""Trainium2 Bass kernel for causal multi-head attention.

Shapes (hardcoded): B=4, T=2048, D=1024, H=16, Dh=64, fp32 I/O.

Strategy (8 NeuronCores, tensor-parallel over heads):
  - Each core c owns heads (2c, 2c+1): computes Q^T/K^T/V projections for its
    128 head-dims over the whole [B*T, D] input (contracting D on the PE),
    then causal flash-style attention in "scores-transposed" orientation
    (S^T[k, q] blocks) so softmax needs no on-chip transposes:
      * exp on ScalarE (no max subtraction: logits are O(+-4) by construction)
      * denominator via an appended ones-column in the V stationary operand
        (partition-axis reduction done by the PE itself)
      * division folded into the PSUM->SBUF copy against a PE-broadcast
        reciprocal
  - An on-device AllToAll re-shards ctx^T from head-sharded to row-sharded,
    then each core computes out rows [1024c : 1024c+1024) = ctx @ Wo + bo.
  - Host side only slices/cats and casts dtypes.

All matmul operands are fp16 (same PE throughput as bf16, 3 extra mantissa
bits); all accumulation is fp32 in PSUM.
"""

import sys

sys.path.insert(0, "/opt/trn_rl_repo")

import numpy as np

import concourse.bass as bass
import concourse.mybir as mybir
import concourse.tile as tile
from concourse import bacc
from concourse import bass_utils

N_CORES = 8
B, T, D, H, DH = 4, 2048, 1024, 16, 64
BT = B * T  # 8192
KS = D // 128  # 8 contraction subtiles
TC = 512  # t-chunk for projections
NTC = BT // TC  # 16
QC = 512  # query chunk in attention
NQC = T // QC  # 4 per batch
KB = 128  # key block
NKB = T // KB  # 16 per batch
ROWS = BT // N_CORES  # 1024 out rows per core

F16 = mybir.dt.float16
F32 = mybir.dt.float32

_CACHE = {}


def _build():
    nc = bacc.Bacc("TRN2", target_bir_lowering=False, num_devices=N_CORES)

    x_d = nc.dram_tensor("x", [D, BT], F16, kind="ExternalInput")  # pre-transposed
    wq_d = nc.dram_tensor("wq", [D, 128], F16, kind="ExternalInput")
    wk_d = nc.dram_tensor("wk", [D, 128], F16, kind="ExternalInput")
    wv_d = nc.dram_tensor("wv", [D, 128], F16, kind="ExternalInput")
    wo_d = nc.dram_tensor("wo", [D, D], F16, kind="ExternalInput")
    bo_d = nc.dram_tensor("bo", [D], F32, kind="ExternalInput")
    e2_d = nc.dram_tensor("e2", [2, 128], F16, kind="ExternalInput")
    cmask_d = nc.dram_tensor("cmask", [4, 128, QC], F16, kind="ExternalInput")
    out_d = nc.dram_tensor("out", [B, ROWS // B, D], F32, kind="ExternalOutput")

    with tile.TileContext(nc) as tc:
        with (
            tc.tile_pool(name="persist", bufs=1) as persist,
            tc.tile_pool(name="xt", bufs=3) as xtp,
            tc.tile_pool(name="work", bufs=12) as work,
            tc.tile_pool(name="tail", bufs=2) as tailp,
            tc.tile_pool(name="ctx", bufs=3) as ctxp,
            tc.tile_pool(name="outp", bufs=3) as outp,
            tc.tile_pool(name="ps_proj", bufs=1, space="PSUM") as ps_proj,
            tc.tile_pool(name="ps_s", bufs=3, space="PSUM") as ps_s,
            tc.tile_pool(name="ps_av", bufs=3, space="PSUM") as ps_av,
            tc.tile_pool(name="ps_rb", bufs=1, space="PSUM") as ps_rb,
            tc.tile_pool(name="dram", bufs=1, space="DRAM") as dram,
        ):
            # ---- persistent state ----
            wq_sb = persist.tile([128, KS, 128], F16)
            wk_sb = persist.tile([128, KS, 128], F16)
            wv_sb = persist.tile([128, KS, 128], F16)
            wo_sb = persist.tile([128, KS, D], F16)
            nc.sync.dma_start(wq_sb[:], wq_d.rearrange("(o p) h -> p o h", p=128))
            nc.sync.dma_start(wk_sb[:], wk_d.rearrange("(o p) h -> p o h", p=128))
            nc.sync.dma_start(wv_sb[:], wv_d.rearrange("(o p) h -> p o h", p=128))
            nc.sync.dma_start(wo_sb[:], wo_d.rearrange("(r p) n -> p r n", p=128))

            qt_sb = persist.tile([128, BT], F16)  # [2 heads x 64, global t]
            kt_sb = persist.tile([128, BT], F16)
            v0_sb = persist.tile([128, B * NKB, DH + 1], F16)  # + ones col
            v1_sb = persist.tile([128, B * NKB, DH + 1], F16)
            nc.vector.memset(v0_sb[:, :, DH : DH + 1], 1.0)
            nc.vector.memset(v1_sb[:, :, DH : DH + 1], 1.0)

            # bias broadcast [128, D] fp32 via PE ones-trick
            ones_col = persist.tile([1, 128], F32)
            nc.vector.memset(ones_col[:], 1.0)
            bo_sb = persist.tile([1, D], F32)
            nc.sync.dma_start(bo_sb[:], bo_d[None, :])
            bias_sb = persist.tile([128, D], F32)
            for nch in range(2):
                bps = ps_proj.tile([128, 512], F32, tag="proj")
                nc.tensor.matmul(
                    bps[:], ones_col[:], bo_sb[:, nch * 512 : (nch + 1) * 512]
                )
                nc.vector.tensor_copy(bias_sb[:, nch * 512 : (nch + 1) * 512], bps[:])

            # E2 selector for reciprocal broadcast: rows 0-63 <- r2[0], 64-127 <- r2[1]
            e2_sb = persist.tile([2, 128], F16)
            nc.sync.dma_start(e2_sb[:], e2_d[:])

            # diagonal causal masks: mask_i[p, j] = 1 if j >= p + i*128 else 0
            cmask_sb = persist.tile([128, 4, QC], F16)
            nc.sync.dma_start(cmask_sb[:], cmask_d.rearrange("i p j -> p i j"))
            masks = [cmask_sb[:, i, :] for i in range(4)]

            # ---- phase 1: projections ----
            for tcn in range(NTC):
                t0 = tcn * TC
                xt = xtp.tile([128, KS, TC], F16, tag="xt")
                nc.sync.dma_start(
                    xt[:],
                    x_d[:, t0 : t0 + TC].rearrange("(o p) t -> p o t", p=128),
                )
                for w_sb, dst in ((wq_sb, qt_sb), (wk_sb, kt_sb)):
                    pp = ps_proj.tile([128, TC], F32, tag="proj")
                    for ks in range(KS):
                        nc.tensor.matmul(
                            pp[:],
                            w_sb[:, ks, :],
                            xt[:, ks, :],
                            start=(ks == 0),
                            stop=(ks == KS - 1),
                        )
                    nc.vector.tensor_copy(dst[:, t0 : t0 + TC], pp[:])
                for sub in range(TC // 128):
                    vp_full = ps_proj.tile([128, TC], F32, tag="proj", name="vp")
                    vp = vp_full[:, :128]
                    for ks in range(KS):
                        nc.tensor.matmul(
                            vp[:],
                            xt[:, ks, sub * 128 : (sub + 1) * 128],
                            wv_sb[:, ks, :],
                            start=(ks == 0),
                            stop=(ks == KS - 1),
                        )
                    kbg = tcn * (TC // 128) + sub
                    nc.vector.tensor_copy(v0_sb[:, kbg, 0:DH], vp[:, 0:DH])
                    nc.vector.tensor_copy(v1_sb[:, kbg, 0:DH], vp[:, DH : 2 * DH])

            # ---- phase 2: attention (scores-transposed flash) ----
            RB4 = ROWS // B  # 256 out rows per core per batch
            cc_ins = [dram.tile([N_CORES, 128, RB4], F16, name=f"cc_in{b}", tag=f"cc_in{b}") for b in range(B)]
            cc_outs = [dram.tile([N_CORES, 128, RB4], F16, name=f"cc_out{b}", tag=f"cc_out{b}") for b in range(B)]
            ao_sbs = []

            for b in range(B):
                for qc in range(NQC):
                    q0 = b * T + qc * QC
                    nkb = 4 * qc + 4
                    av0_full = ps_av.tile([128, QC], F32, tag="av", name="av0")
                    av1_full = ps_av.tile([128, QC], F32, tag="av", name="av1")
                    av0 = av0_full[: DH + 1]
                    av1 = av1_full[: DH + 1]
                    for kb in range(nkb):
                        k0 = b * T + kb * KB
                        kbg = b * NKB + kb
                        first, last = kb == 0, kb == nkb - 1
                        for h, av in ((0, av0), (1, av1)):
                            hs = slice(h * 64, (h + 1) * 64)
                            sp = ps_s.tile([128, QC], F32, tag="s")
                            nc.tensor.matmul(
                                sp[:],
                                kt_sb[hs, k0 : k0 + KB],
                                qt_sb[hs, q0 : q0 + QC],
                            )
                            e = work.tile([128, QC], F16, tag="e")
                            nc.scalar.activation(
                                e[:], sp[:], mybir.ActivationFunctionType.Exp,
                                scale=0.125,
                            )
                            if kb >= 4 * qc:
                                nc.vector.tensor_tensor(
                                    e[:], e[:], masks[kb - 4 * qc],
                                    mybir.AluOpType.mult,
                                )
                            vsb = v0_sb if h == 0 else v1_sb
                            nc.tensor.matmul(
                                av[:], vsb[:, kbg, :], e[:],
                                start=first, stop=last,
                            )
                    # reciprocal of denominators (row 64 of each av bank)
                    u0 = tailp.tile([DH + 1, QC], F32, tag="u0")
                    u1 = tailp.tile([DH + 1, QC], F32, tag="u1")
                    nc.vector.tensor_copy(u0[:], av0[:])
                    nc.vector.tensor_copy(u1[:], av1[:])
                    d2a = tailp.tile([1, QC], F32, tag="d2a")
                    d2b = tailp.tile([1, QC], F32, tag="d2b")
                    nc.vector.tensor_copy(d2a[:], av0[DH : DH + 1, :])
                    nc.vector.tensor_copy(d2b[:], av1[DH : DH + 1, :])
                    r2 = tailp.tile([2, QC], F32, tag="r2")
                    r1t = tailp.tile([1, QC], F32, tag="r1t")
                    nc.vector.reciprocal_approx_fast(r2[0:1, :], d2a[:])
                    nc.vector.reciprocal_approx_fast(r1t[:], d2b[:])
                    nc.sync.dma_start(r2[1:2, :], r1t[:])
                    r2h = tailp.tile([2, QC], F16, tag="r2h")
                    nc.vector.tensor_copy(r2h[:], r2[:])
                    rb = ps_rb.tile([128, QC], F32, tag="rb")
                    nc.tensor.matmul(rb[:], e2_sb[:], r2h[:])
                    ctx2 = ctxp.tile([128, QC], F16, tag="ctx")
                    nc.vector.tensor_tensor(
                        ctx2[0:64, :], u0[0:64, :], rb[0:64, :],
                        mybir.AluOpType.mult,
                    )
                    nc.vector.tensor_tensor(
                        ctx2[64:128, :], u1[0:64, :], rb[64:128, :],
                        mybir.AluOpType.mult,
                    )
                    s0 = qc * QC // RB4  # first shard covered by this chunk
                    nc.sync.dma_start(
                        cc_ins[b][s0 : s0 + QC // RB4].rearrange("s p f -> p s f"),
                        ctx2[:].rearrange("p (s f) -> p s f", s=QC // RB4),
                    )

                # ---- per-batch all-to-all + output projection ----
                nc.gpsimd.collective_compute(
                    "AllToAll",
                    mybir.AluOpType.bypass,
                    replica_groups=[list(range(N_CORES))],
                    ins=[cc_ins[b][:]],
                    outs=[cc_outs[b][:]],
                )
                ao_sb = persist.tile([128, KS, RB4], F16, name=f"ao{b}", tag=f"ao{b}")
                ao_sbs.append(ao_sb)
                nc.sync.dma_start(ao_sb[:], cc_outs[b].rearrange("r p t -> p r t"))
                for mb in range(RB4 // 128):
                    for nch in range(2):
                        op = ps_proj.tile([128, 512], F32, tag="proj")
                        for r in range(KS):
                            nc.tensor.matmul(
                                op[:],
                                ao_sb[:, r, mb * 128 : (mb + 1) * 128],
                                wo_sb[:, r, nch * 512 : (nch + 1) * 512],
                                start=(r == 0),
                                stop=(r == KS - 1),
                            )
                        osb = outp.tile([128, 512], F32, tag="osb")
                        nc.vector.tensor_tensor(
                            osb[:], op[:], bias_sb[:, nch * 512 : (nch + 1) * 512],
                            mybir.AluOpType.add,
                        )
                        nc.sync.dma_start(
                            out_d[b, mb * 128 : (mb + 1) * 128,
                                  nch * 512 : (nch + 1) * 512],
                            osb[:],
                        )

    nc.compile()
    return nc


def _get_nc():
    if "nc" not in _CACHE:
        _CACHE["nc"] = _build()
    return _CACHE["nc"]


def prepare_in_maps(x, Wq, Wk, Wv, Wo, bo):
    x16 = np.ascontiguousarray(np.asarray(x, dtype=np.float32).reshape(BT, D).T).astype(np.float16)
    wo16 = np.asarray(Wo, dtype=np.float32).astype(np.float16)
    bo32 = np.ascontiguousarray(np.asarray(bo, dtype=np.float32))
    e2 = np.zeros((2, 128), dtype=np.float16)
    e2[0, 0:64] = 1.0
    e2[1, 64:128] = 1.0
    cmask = np.zeros((4, 128, QC), dtype=np.float16)
    for i in range(4):
        p = np.arange(128)[:, None]
        j = np.arange(QC)[None, :]
        cmask[i] = (j >= p + i * 128).astype(np.float16)
    in_maps = []
    for c in range(N_CORES):
        cs = slice(128 * c, 128 * (c + 1))
        in_maps.append(
            {
                "x": x16,
                "wq": np.ascontiguousarray(np.asarray(Wq, np.float32)[:, cs]).astype(np.float16),
                "wk": np.ascontiguousarray(np.asarray(Wk, np.float32)[:, cs]).astype(np.float16),
                "wv": np.ascontiguousarray(np.asarray(Wv, np.float32)[:, cs]).astype(np.float16),
                "wo": wo16,
                "bo": bo32,
                "e2": e2,
                "cmask": cmask,
            }
        )
    return in_maps


def kernel(x, Wq, Wk, Wv, Wo, bo, _trace=False):
    nc = _get_nc()
    in_maps = prepare_in_maps(x, Wq, Wk, Wv, Wo, bo)
    res = bass_utils.run_bass_kernel_spmd(
        nc, in_maps, list(range(N_CORES)), trace=_trace
    )
    if _trace:
        _CACHE["last_results"] = res
    out = np.empty((B, T, D), dtype=np.float32)
    rb4 = ROWS // B
    for c in range(N_CORES):
        oc = res.results[c]["out"]  # [B, 256, D]
        for b in range(B):
            out[b, rb4 * c : rb4 * (c + 1), :] = oc[b]
    return out
